# revision 2
# baseline (speedup 1.0000x reference)
"""GridMask apply (BatchHide): out = feature * mask, mask broadcast over channels.

feature: [32, 128, 224, 224] f32, mask: [32, 1, 224, 224] f32.
Data-parallel over batch across 8 NeuronCores (4 samples per core).

bf16 I/O: feature/mask are converted to bf16 on the host before upload, the
device multiplies in bf16 and writes bf16, and the host upconverts the result
to f32. Halves HBM traffic vs f32; worst-case elementwise rel err is ~2^-9,
far inside the 2e-2 gate. The kernel is HBM-bound either way, so dtype is the
dominant lever.

Per-core layout: partition dim = (cpg channel-reps) x (g hw-groups), free dim =
(m channel-repeats) x (t hw elems), with cpg*g = 128, m*cpg = ct (channels per
tile), g*t = HW. Contiguous DRAM run per DMA descriptor = t*2 bytes. The mask
tile [128, t] shares the partition mapping of every channel's feature tile, so
one mask load per sample serves all channels via a stride-0 free-dim broadcast.

Measured on trn2 (g=32, ct=32, separate load/store rings): ~286 us/core, ~99%
of the ~368 GB/s per-core HBM roofline for the ~104 MB/core of traffic.
Loads ride the sync HWDGE ring and stores the scalar ring exclusively —
interleaving them in one FIFO ring head-of-line-blocks loads behind stores
that wait on their tile's multiply (~46 us penalty).
"""

import numpy as np
import ml_dtypes

import concourse.bacc as bacc
import concourse.tile as tile
from concourse import mybir
from concourse.bass_utils import run_bass_kernel_spmd

B, C, H, W = 32, 128, 224, 224
N_CORES = 8
B_LOC = B // N_CORES  # 4 samples per core
HW = H * W  # 50176
P = 128
BF16 = mybir.dt.bfloat16

_nc_cache = {}


def _build(g=32, ct=32, bufs=6, dual_ring=False, split_mul=False):
    """g: hw-groups on the partition dim; cpg = 128//g channel-reps fill the rest.
    ct: channels per tile. Contiguous DRAM run per descriptor = (HW//g)*2 bytes.
    """
    cpg = P // g  # channel-reps on the partition dim
    m = ct // cpg  # channel repeats along the free dim
    t = HW // g  # hw elems per partition chunk
    assert cpg * m == ct and g * t == HW and C % ct == 0

    nc = bacc.Bacc("TRN2", target_bir_lowering=False, debug=False, num_devices=N_CORES)
    feat = nc.dram_tensor("feature", [B_LOC, C, HW], BF16, kind="ExternalInput").ap()
    msk = nc.dram_tensor("mask", [B_LOC, HW], BF16, kind="ExternalInput").ap()
    out = nc.dram_tensor("out", [B_LOC, C, HW], BF16, kind="ExternalOutput").ap()

    # Channel-tile widths per batch: taper the first tiles of batch 0 (start
    # compute sooner) and the last tiles of the final batch (shorter drain).
    def widths(b):
        w = [ct] * (C // ct)
        if ct >= 4 * cpg:
            q = ct // 4
            if b == 0:
                w = [q, q, q, q] + w[1:]
            if b == B_LOC - 1:
                w = w[:-1] + [q, q, q, q]
        assert sum(w) == C and all(x % cpg == 0 for x in w)
        return w

    with tile.TileContext(nc) as tc:
        with (
            tc.tile_pool(name="mask", bufs=B_LOC) as mpool,
            tc.tile_pool(name="data", bufs=bufs) as dpool,
        ):
            # All masks upfront on the (initially idle) scalar ring.
            mts = []
            for b in range(B_LOC):
                mt = mpool.tile([P, t], BF16)
                mbc = msk[b].rearrange("(g t) -> g t", g=g)[None, :, :].broadcast_to(
                    [cpg, g, t]
                )
                nc.scalar.dma_start(out=mt[:], in_=mbc)
                mts.append(mt)
            it = 0
            for b in range(B_LOC):
                mt = mts[b]
                for w, c0 in zip(widths(b), np.cumsum([0] + widths(b)[:-1])):
                    c0 = int(c0)
                    mi = w // cpg  # channel repeats along free dim for this tile
                    fv = feat[b, c0 : c0 + w].rearrange(
                        "(m cg) (g t) -> (cg g) m t", cg=cpg, g=g
                    )
                    ov = out[b, c0 : c0 + w].rearrange(
                        "(m cg) (g t) -> (cg g) m t", cg=cpg, g=g
                    )
                    if dual_ring and it % 2 == 1:
                        ld, st = nc.scalar, nc.sync
                    else:
                        ld, st = nc.sync, nc.scalar
                    it += 1
                    ft = dpool.tile([P, m, t], BF16, tag="data")
                    nc_ft = ft[:, :mi, :]
                    ld.dma_start(out=nc_ft, in_=fv)
                    if split_mul:
                        for j in range(mi):
                            nc.vector.tensor_mul(
                                out=ft[:, j, :], in0=ft[:, j, :], in1=mt[:]
                            )
                    else:
                        nc.vector.tensor_mul(
                            out=nc_ft,
                            in0=nc_ft,
                            in1=mt[:, None, :].broadcast_to([P, mi, t]),
                        )
                    st.dma_start(out=ov, in_=nc_ft)
    nc.compile()
    return nc


def _get_nc(**kw):
    key = tuple(sorted(kw.items()))
    if key not in _nc_cache:
        _nc_cache[key] = _build(**kw)
    return _nc_cache[key]


def _in_maps(feature, mask):
    fb = np.ascontiguousarray(np.asarray(feature)).astype(ml_dtypes.bfloat16)
    mb = np.ascontiguousarray(np.asarray(mask)).astype(ml_dtypes.bfloat16)
    return [
        {
            "feature": fb[i * B_LOC : (i + 1) * B_LOC].reshape(B_LOC, C, HW),
            "mask": mb[i * B_LOC : (i + 1) * B_LOC].reshape(B_LOC, HW),
        }
        for i in range(N_CORES)
    ]


def kernel(feature, mask, **cfg):
    nc = _get_nc(**cfg)
    res = run_bass_kernel_spmd(nc, _in_maps(feature, mask), list(range(N_CORES))).results
    return np.concatenate(
        [
            res[i]["out"].astype(np.float32).reshape(B_LOC, C, H, W)
            for i in range(N_CORES)
        ],
        axis=0,
    )


# revision 4
# speedup vs baseline: 1.0868x; 1.0868x over previous
"""GridMask apply (BatchHide): out = feature * mask, mask broadcast over channels.

feature: [32, 128, 224, 224] f32, mask: [32, 1, 224, 224] f32.
Data-parallel over batch across 8 NeuronCores (4 samples per core).

bf16 I/O: feature/mask are converted to bf16 on the host before upload, the
device multiplies in bf16 and writes bf16, and the host upconverts the result
to f32. Halves HBM traffic vs f32; worst-case elementwise rel err is ~2^-9,
far inside the 2e-2 gate. The kernel is HBM-bound either way, so dtype is the
dominant lever.

Per-core layout: partition dim = (cpg channel-reps) x (g hw-groups), free dim =
(m channel-repeats) x (t hw elems), with cpg*g = 128, m*cpg = ct (channels per
tile), g*t = HW. Contiguous DRAM run per DMA descriptor = t*2 bytes. The mask
tile [128, t] shares the partition mapping of every channel's feature tile, so
one mask load per sample serves all channels via a stride-0 free-dim broadcast.

Measured on trn2 (g=32, ct=16, bufs=12, separate load/store rings): ~283
us/core, ~99% of the ~368 GB/s per-core HBM roofline for the ~104 MB/core of
traffic. Loads ride the sync HWDGE ring and stores the scalar ring
exclusively — interleaving them in one FIFO ring head-of-line-blocks loads
behind stores that wait on their tile's multiply (~46 us penalty). Deep
buffering (12 tiles in flight) rides out HBM-contention latency spikes from
co-tenants.
"""

import numpy as np
import ml_dtypes

import concourse.bacc as bacc
import concourse.tile as tile
from concourse import mybir
from concourse.bass_utils import run_bass_kernel_spmd

B, C, H, W = 32, 128, 224, 224
N_CORES = 8
B_LOC = B // N_CORES  # 4 samples per core
HW = H * W  # 50176
P = 128
BF16 = mybir.dt.bfloat16

_nc_cache = {}


def _build(g=32, ct=16, bufs=12, dual_ring=False, split_mul=False):
    """g: hw-groups on the partition dim; cpg = 128//g channel-reps fill the rest.
    ct: channels per tile. Contiguous DRAM run per descriptor = (HW//g)*2 bytes.
    """
    cpg = P // g  # channel-reps on the partition dim
    m = ct // cpg  # channel repeats along the free dim
    t = HW // g  # hw elems per partition chunk
    assert cpg * m == ct and g * t == HW and C % ct == 0

    nc = bacc.Bacc("TRN2", target_bir_lowering=False, debug=False, num_devices=N_CORES)
    feat = nc.dram_tensor("feature", [B_LOC, C, HW], BF16, kind="ExternalInput").ap()
    msk = nc.dram_tensor("mask", [B_LOC, HW], BF16, kind="ExternalInput").ap()
    out = nc.dram_tensor("out", [B_LOC, C, HW], BF16, kind="ExternalOutput").ap()

    # Channel-tile widths per batch: taper the first tiles of batch 0 (start
    # compute sooner) and the last tiles of the final batch (shorter drain).
    def widths(b):
        w = [ct] * (C // ct)
        if ct >= 4 * cpg:
            q = ct // 4
            if b == 0:
                w = [q, q, q, q] + w[1:]
            if b == B_LOC - 1:
                w = w[:-1] + [q, q, q, q]
        assert sum(w) == C and all(x % cpg == 0 for x in w)
        return w

    with tile.TileContext(nc) as tc:
        with (
            tc.tile_pool(name="mask", bufs=B_LOC) as mpool,
            tc.tile_pool(name="data", bufs=bufs) as dpool,
        ):
            # All masks upfront on the (initially idle) scalar ring.
            mts = []
            for b in range(B_LOC):
                mt = mpool.tile([P, t], BF16)
                mbc = msk[b].rearrange("(g t) -> g t", g=g)[None, :, :].broadcast_to(
                    [cpg, g, t]
                )
                nc.scalar.dma_start(out=mt[:], in_=mbc)
                mts.append(mt)
            it = 0
            for b in range(B_LOC):
                mt = mts[b]
                for w, c0 in zip(widths(b), np.cumsum([0] + widths(b)[:-1])):
                    c0 = int(c0)
                    mi = w // cpg  # channel repeats along free dim for this tile
                    fv = feat[b, c0 : c0 + w].rearrange(
                        "(m cg) (g t) -> (cg g) m t", cg=cpg, g=g
                    )
                    ov = out[b, c0 : c0 + w].rearrange(
                        "(m cg) (g t) -> (cg g) m t", cg=cpg, g=g
                    )
                    if dual_ring and it % 2 == 1:
                        ld, st = nc.scalar, nc.sync
                    else:
                        ld, st = nc.sync, nc.scalar
                    it += 1
                    ft = dpool.tile([P, m, t], BF16, tag="data")
                    nc_ft = ft[:, :mi, :]
                    ld.dma_start(out=nc_ft, in_=fv)
                    if split_mul:
                        for j in range(mi):
                            nc.vector.tensor_mul(
                                out=ft[:, j, :], in0=ft[:, j, :], in1=mt[:]
                            )
                    else:
                        nc.vector.tensor_mul(
                            out=nc_ft,
                            in0=nc_ft,
                            in1=mt[:, None, :].broadcast_to([P, mi, t]),
                        )
                    st.dma_start(out=ov, in_=nc_ft)
    nc.compile()
    return nc


def _get_nc(**kw):
    key = tuple(sorted(kw.items()))
    if key not in _nc_cache:
        _nc_cache[key] = _build(**kw)
    return _nc_cache[key]


def _in_maps(feature, mask):
    fb = np.ascontiguousarray(np.asarray(feature)).astype(ml_dtypes.bfloat16)
    mb = np.ascontiguousarray(np.asarray(mask)).astype(ml_dtypes.bfloat16)
    return [
        {
            "feature": fb[i * B_LOC : (i + 1) * B_LOC].reshape(B_LOC, C, HW),
            "mask": mb[i * B_LOC : (i + 1) * B_LOC].reshape(B_LOC, HW),
        }
        for i in range(N_CORES)
    ]


def kernel(feature, mask, **cfg):
    nc = _get_nc(**cfg)
    res = run_bass_kernel_spmd(nc, _in_maps(feature, mask), list(range(N_CORES))).results
    return np.concatenate(
        [
            res[i]["out"].astype(np.float32).reshape(B_LOC, C, H, W)
            for i in range(N_CORES)
        ],
        axis=0,
    )


# revision 9
# speedup vs baseline: 1.2481x; 1.1484x over previous
"""GridMask apply (BatchHide): out = feature * mask, mask broadcast over channels.

feature: [32, 128, 224, 224] f32, mask: [32, 1, 224, 224] f32.
Data-parallel over batch across 8 NeuronCores (4 samples per core).

bf16 I/O: feature/mask are converted to bf16 on the host before upload, the
device multiplies in bf16 and writes bf16, and the host upconverts the result
to f32. Halves HBM traffic vs f32; worst-case elementwise rel err is ~2^-9,
far inside the 2e-2 gate. The kernel is HBM-bound either way, so dtype is the
dominant lever.

Per-core layout: partition dim = (cpg channel-reps) x (g hw-groups), free dim =
(m channel-repeats) x (t hw elems), with cpg*g = 128, m*cpg = ct (channels per
tile), g*t = HW. Contiguous DRAM run per DMA descriptor = t*2 bytes. The mask
tile [128, t] shares the partition mapping of every channel's feature tile, so
one mask load per sample serves all channels via a stride-0 free-dim broadcast.

Measured on trn2 (g=32, ct=16, bufs=12, separate load/store rings, fp8 mask
split per partition group): ~267 us/core, ~386 GB/s effective for the
~103 MB/core of traffic — at/above the per-core HBM roofline. Loads ride the
sync HWDGE ring and stores the scalar ring exclusively — interleaving them in
one FIFO ring head-of-line-blocks loads behind stores that wait on their
tile's multiply (~46 us penalty). Deep buffering (12 tiles in flight) rides
out HBM-contention latency spikes from co-tenants. The mask is uploaded as
fp8 (0/1 exact) and split into per-replica DMAs: a single broadcast mask DMA
lands all its descriptors on DMA engines 0-3 (~17 us hotspot).
"""

import numpy as np
import ml_dtypes

import concourse.bacc as bacc
import concourse.tile as tile
from concourse import mybir
from concourse.bass_utils import run_bass_kernel_spmd

B, C, H, W = 32, 128, 224, 224
N_CORES = 8
B_LOC = B // N_CORES  # 4 samples per core
HW = H * W  # 50176
P = 128
BF16 = mybir.dt.bfloat16

_nc_cache = {}


def _build(
    g=32, ct=16, bufs=12, dual_ring=False, split_mul=False, mask_fp8=True,
    mask_split=True,
):
    """g: hw-groups on the partition dim; cpg = 128//g channel-reps fill the rest.
    ct: channels per tile. Contiguous DRAM run per descriptor = (HW//g)*2 bytes.
    """
    cpg = P // g  # channel-reps on the partition dim
    m = ct // cpg  # channel repeats along the free dim
    t = HW // g  # hw elems per partition chunk
    assert cpg * m == ct and g * t == HW and C % ct == 0

    nc = bacc.Bacc("TRN2", target_bir_lowering=False, debug=False, num_devices=N_CORES)
    feat = nc.dram_tensor("feature", [B_LOC, C, HW], BF16, kind="ExternalInput").ap()
    mdt = mybir.dt.float8e4 if mask_fp8 else BF16
    msk = nc.dram_tensor("mask", [B_LOC, HW], mdt, kind="ExternalInput").ap()
    out = nc.dram_tensor("out", [B_LOC, C, HW], BF16, kind="ExternalOutput").ap()

    # Channel-tile widths per batch: taper the first tiles of batch 0 (start
    # compute sooner) and the last tiles of the final batch (shorter drain).
    def widths(b):
        w = [ct] * (C // ct)
        if ct >= 4 * cpg:
            q = ct // 4
            if b == 0:
                w = [q, q, q, q] + w[1:]
            if b == B_LOC - 1:
                w = w[:-1] + [q, q, q, q]
        assert sum(w) == C and all(x % cpg == 0 for x in w)
        return w

    with tile.TileContext(nc) as tc:
        with (
            tc.tile_pool(name="mask", bufs=B_LOC) as mpool,
            tc.tile_pool(name="data", bufs=bufs) as dpool,
        ):
            # All masks upfront. The mask is 0/1-valued, exactly representable
            # in fp8e4; the fp8->bf16 cast rides the SWDGE (gpsimd) DMA path.
            # mask_split issues one DMA per channel-replica partition group: a
            # single broadcast DMA puts all its descriptors on DMA engines 0-3,
            # adding ~17us of serialized store-ring work to those four engines;
            # split DMAs spread descriptors across all 16 engines.
            mts = []
            for b in range(B_LOC):
                mt = mpool.tile([P, t], BF16)
                meng = nc.gpsimd if mask_fp8 else nc.scalar
                mv = msk[b].rearrange("(g t) -> g t", g=g)
                if mask_split:
                    for r in range(cpg):
                        meng.dma_start(out=mt[r * g : (r + 1) * g, :], in_=mv)
                else:
                    meng.dma_start(
                        out=mt[:], in_=mv[None, :, :].broadcast_to([cpg, g, t])
                    )
                mts.append(mt)
            it = 0
            for b in range(B_LOC):
                mt = mts[b]
                for w, c0 in zip(widths(b), np.cumsum([0] + widths(b)[:-1])):
                    c0 = int(c0)
                    mi = w // cpg  # channel repeats along free dim for this tile
                    fv = feat[b, c0 : c0 + w].rearrange(
                        "(m cg) (g t) -> (cg g) m t", cg=cpg, g=g
                    )
                    ov = out[b, c0 : c0 + w].rearrange(
                        "(m cg) (g t) -> (cg g) m t", cg=cpg, g=g
                    )
                    if dual_ring and it % 2 == 1:
                        ld, st = nc.scalar, nc.sync
                    else:
                        ld, st = nc.sync, nc.scalar
                    it += 1
                    ft = dpool.tile([P, m, t], BF16, tag="data")
                    nc_ft = ft[:, :mi, :]
                    ld.dma_start(out=nc_ft, in_=fv)
                    if split_mul:
                        for j in range(mi):
                            nc.vector.tensor_mul(
                                out=ft[:, j, :], in0=ft[:, j, :], in1=mt[:]
                            )
                    else:
                        nc.vector.tensor_mul(
                            out=nc_ft,
                            in0=nc_ft,
                            in1=mt[:, None, :].broadcast_to([P, mi, t]),
                        )
                    st.dma_start(out=ov, in_=nc_ft)
    nc.compile()
    return nc


def _get_nc(**kw):
    key = tuple(sorted(kw.items()))
    if key not in _nc_cache:
        _nc_cache[key] = _build(**kw)
    return _nc_cache[key]


def _in_maps(feature, mask, mask_fp8=True):
    fb = np.ascontiguousarray(np.asarray(feature)).astype(ml_dtypes.bfloat16)
    mdt = ml_dtypes.float8_e4m3 if mask_fp8 else ml_dtypes.bfloat16
    mb = np.ascontiguousarray(np.asarray(mask)).astype(mdt)
    return [
        {
            "feature": fb[i * B_LOC : (i + 1) * B_LOC].reshape(B_LOC, C, HW),
            "mask": mb[i * B_LOC : (i + 1) * B_LOC].reshape(B_LOC, HW),
        }
        for i in range(N_CORES)
    ]


def kernel(feature, mask, **cfg):
    nc = _get_nc(**cfg)
    res = run_bass_kernel_spmd(
        nc, _in_maps(feature, mask, cfg.get("mask_fp8", True)), list(range(N_CORES))
    ).results
    return np.concatenate(
        [
            res[i]["out"].astype(np.float32).reshape(B_LOC, C, H, W)
            for i in range(N_CORES)
        ],
        axis=0,
    )
